# revision 21
# baseline (speedup 1.0000x reference)
"""Trainium2 Bass kernel for the Binary-MLP (nn_Binary0) problem.

Strategy (8-way batch-parallel, 1024 rows/core):
  fc1: h1 = x @ sign(w1).T        -- fp32r 2-pass split of x (hi = RNE to
       a1 = sign(h1 - t1)            12 significand bits, the HW-measured
                                     fp32r read width, so hi passes exact;
                                     lo residual <= 12 bits). +-1 weights
                                     shipped fp8, converted on-device.
  fc2: h2 = a1 @ sign(w2).T        -- fp8 DoubleRow (exact: +-1 products,
       a2 = sign(h2 - t2)            fp32 psum)
  fc3: h3 = a2 @ sign(w3).T        -- fp8 DoubleRow
       h3c = clip(h3*s3 + c3, -1, 1)
  fc4: logits.T = w4 @ h3c         -- fp32r, fused into fc3 loop; 4 column
                                      groups of the PE array run the 4
                                      M=16 matmuls of a slab concurrently,
                                      stripes summed by a selection matmul
  out = log_softmax(logits)        -- PE-transpose to [batch, cls]

All activations live feature-major [feature, batch] so per-feature
thresholds are per-partition ACT bias vectors, and each layer's sign
outputs land directly in the DoubleRow-paired [k, 2, batch] slab layout
the next layer needs.  A short block of dummy matmuls at kernel start
keeps the PE busy through the DMA ramp so the HAM clock-gate is already
released (2.4 GHz) when the real matmul stream begins.
"""
import sys

for _p in ("/opt/trn_rl_repo",):
    if _p not in sys.path:
        sys.path.insert(0, _p)

import numpy as np

import concourse.bass as bass
import concourse.tile as tile
import concourse.mybir as mybir
from concourse.bass_utils import run_bass_kernel_spmd
from concourse.masks import make_identity

F32 = mybir.dt.float32
F32R = mybir.dt.float32r
BF16 = mybir.dt.bfloat16
FP8 = mybir.dt.float8e4
NP_FP8 = mybir.dt.np(FP8)

EPS = 1e-5
NCORES = 8
B = 8192
BC = B // NCORES            # 1024 batch rows per core
D0, D1, D2 = 784, 3072, 6144
K1F = 6                     # full 128-row k-tiles of the 784-dim input
NXT = 13                    # xr sbuf tiles: 6 hi + 6 lo + 1 packed remainder
NJ1 = D1 // 128             # 24 fc1 output feature tiles
NT2 = D1 // 256             # 12 fc2 DoubleRow contraction tiles
NJ2 = D2 // 128             # 48
NT3 = D2 // 256             # 24 fc3 DoubleRow contraction tiles
NJ3 = D2 // 128             # 48
JB = 4                      # j-tiles per streamed weight slab
NB = 2                      # 512-wide batch halves of BC
NBCH = BC // 128            # 8 batch chunks
NCLS = 16                   # padded class dim (10 real)
NWARM = 10                  # PE warm-up matmuls (HAM release during DMA ramp)

TRACE = False               # test.py sets True for profiling
TRACE_DIR = None
LAST_EXEC_NS = None

DR = mybir.MatmulPerfMode.DoubleRow
ACTF = mybir.ActivationFunctionType
ALU = mybir.AluOpType


def _legalize_multiwait(nc):
    """This container's walrus build rejects >1 sync-wait on one instruction
    (codegen 'Too many sync wait commands'); split extra waits into NoOps."""
    n = 0
    for f in nc.m.functions:
        for blk in f.blocks:
            insts = list(blk.instructions)
            new = []
            changed = False
            for ins in insts:
                si = ins.sync_info
                waits = list(si.on_wait) if (si is not None and si.on_wait) else []
                if len(waits) > 1:
                    for k, w in enumerate(waits[:-1]):
                        nop = mybir.InstNoOp(name=f"{ins.name}-sw{k}", ins=[], outs=[])
                        nop.engine = ins.engine
                        nop.sync_info = mybir.SyncInfo(on_wait=[w], on_update=[])
                        new.append(nop)
                        n += 1
                    ins.sync_info = mybir.SyncInfo(
                        on_wait=[waits[-1]], on_update=list(si.on_update or [])
                    )
                    changed = True
                new.append(ins)
            if changed:
                blk.instructions = new
    return n


def _build_nc():
    nc = bass.Bass("TRN2")

    xrt = nc.dram_tensor("xrt", [NXT, 128, BC], F32R, kind="ExternalInput")
    w1t = nc.dram_tensor("w1t", [K1F + 1, 128, D1], FP8, kind="ExternalInput")
    w2p = nc.dram_tensor("w2p", [NJ2 // JB, NT2, 128, 2, JB * 128], FP8,
                         kind="ExternalInput")
    w3p = nc.dram_tensor("w3p", [NJ3 // JB, NT3, 128, 2, JB * 128], FP8,
                         kind="ExternalInput")
    # per j-slot 32 cols: w4 bf16 hi part in 0:16, lo part in 16:32 (the
    # stripe-sum selection matmul recombines hi+lo for fp32-level accuracy)
    w4t = nc.dram_tensor("w4t", [128, NJ3 * 2 * NCLS], BF16,
                         kind="ExternalInput")
    # cvec columns: [0:24]=-t1, [24:72]=-t2, [72:120]=s3, [120:168]=c3,
    # [168:184] = stripe/hi-lo selection matrix for the fc4 reduction
    cvec = nc.dram_tensor("cvec", [128, NJ1 + 3 * NJ3 + NCLS], F32,
                          kind="ExternalInput")
    b4c = nc.dram_tensor("b4c", [NCLS, 1], F32, kind="ExternalInput")
    out = nc.dram_tensor("out", [BC, 10], F32, kind="ExternalOutput")

    with tile.TileContext(nc) as tc:
        with (
            tc.tile_pool(name="consts", bufs=1) as consts,
            tc.tile_pool(name="a1p", bufs=1) as a1p,
            tc.tile_pool(name="psum", bufs=5, space="PSUM") as psum,
            tc.tile_pool(name="psum_lg", bufs=2, space="PSUM") as psum_lg,
            tc.tile_pool(name="psum_tp", bufs=1, space="PSUM") as psum_tp,
        ):
            a1 = a1p.tile([128, NT2, 2, BC], FP8)

            # fc4 logits accumulators, 4 class-group stripes at partitions
            # {0,32,64,96}+0:16, pre-zeroed, accumulated with start=False
            lg = [psum_lg.tile([128, 512], F32, tag="lg", name=f"lg{i}")
                  for i in range(NB)]

            # ---- PE warm-up: dummy matmuls on a zeroed a1 corner keep the
            # PE busy through the DMA ramp so HAM releases the clock gate
            # before the real stream starts (that a1 region is rewritten by
            # fc1's last sign outputs, long after the warm-up drains).
            nc.vector.memset(a1[:, 11, :, 512:1024], 0.0)
            wps = psum.tile([128, 512], F32, tag="ps")
            for _ in range(NWARM):
                nc.tensor.matmul(wps, lhsT=a1[:, 11, 0, 512:640],
                                 rhs=a1[:, 11, 1, 512:1024],
                                 start=True, stop=True)

            # ---- fc1: fp32r 2-pass exact fp32 matmul + sign threshold ----
            # xr tiles 0..5: hi k-tiles, 6..11: lo k-tiles, 12: packed
            # remainders (hi rows 768..783 at partitions 0:16, lo at 16:32).
            # w1 ships fp8 (DMA on the scalar queue), DVE-converts to f32r.
            with (
                tc.tile_pool(name="fc1res", bufs=1) as fc1res,
                tc.tile_pool(name="w1f8", bufs=4) as w1f8,
            ):
                xr = fc1res.tile([128, NXT, BC], F32R)
                w1 = fc1res.tile([128, K1F + 1, D1], F32R)
                for cb in range(3):
                    c = cb * 1024
                    for k in range(K1F + 1):
                        ch = w1f8.tile([128, 1024], FP8, tag="w1c")
                        if k < K1F:
                            nc.scalar.dma_start(out=ch,
                                                in_=w1t[k][:, c:c + 1024])
                            nc.vector.tensor_copy(out=w1[:, k, c:c + 1024],
                                                  in_=ch)
                        else:
                            nc.scalar.dma_start(out=ch[0:32, :],
                                                in_=w1t[K1F][0:32, c:c + 1024])
                            nc.vector.tensor_copy(out=w1[0:32, K1F, c:c + 1024],
                                                  in_=ch[0:32, :])
                        if cb == 0:
                            if k < K1F:
                                nc.sync.dma_start(out=xr[:, k, 0:512],
                                                  in_=xrt[k][:, 0:512])
                                nc.sync.dma_start(out=xr[:, K1F + k, 0:512],
                                                  in_=xrt[K1F + k][:, 0:512])
                            else:
                                nc.sync.dma_start(out=xr[0:32, 12, 0:512],
                                                  in_=xrt[12][0:32, 0:512])
                            if k == 1:
                                cv = consts.tile([128, NJ1 + 3 * NJ3 + NCLS],
                                                 F32)
                                nc.sync.dma_start(out=cv, in_=cvec[:, :])
                                nt1 = cv[:, 0:NJ1]
                                nt2 = cv[:, NJ1:NJ1 + NJ3]
                                s3s = cv[:, NJ1 + NJ3:NJ1 + 2 * NJ3]
                                c3s = cv[:, NJ1 + 2 * NJ3:NJ1 + 3 * NJ3]
                                selm = cv[:, NJ1 + 3 * NJ3:]
                                w4s = consts.tile([128, NJ3, 2 * NCLS], BF16)
                                nc.sync.dma_start(
                                    out=w4s,
                                    in_=w4t.rearrange("p (j c) -> p j c",
                                                      c=2 * NCLS))
                                b4s = consts.tile([NCLS, 1], F32)
                                nc.sync.dma_start(out=b4s, in_=b4c[:, :])
                                for n in range(NB):
                                    nc.vector.memset(lg[n], 0.0)
                for i in range(NXT - 1):
                    nc.sync.dma_start(out=xr[:, i, 512:1024],
                                      in_=xrt[i][:, 512:1024])
                nc.sync.dma_start(out=xr[0:32, 12, 512:1024],
                                  in_=xrt[12][0:32, 512:1024])

                # j-pairs: two psum groups share the moving stream so each
                # fp32 LDWEIGHTS hides under the other tile's matmuls
                for n in range(NB):
                    ns = slice(n * 512, (n + 1) * 512)
                    for jp in range(NJ1 // 2):
                        ja, jb_ = 2 * jp, 2 * jp + 1
                        ps0 = psum.tile([128, 512], F32, tag="ps")
                        ps1 = psum.tile([128, 512], F32, tag="ps")
                        for k in range(K1F):
                            wa = w1[:, k, ja * 128:(ja + 1) * 128]
                            wb = w1[:, k, jb_ * 128:(jb_ + 1) * 128]
                            nc.tensor.matmul(ps0, lhsT=wa, rhs=xr[:, k, ns],
                                             start=(k == 0), stop=False)
                            nc.tensor.matmul(ps0, lhsT=wa,
                                             rhs=xr[:, K1F + k, ns],
                                             start=False, stop=False)
                            nc.tensor.matmul(ps1, lhsT=wb, rhs=xr[:, k, ns],
                                             start=(k == 0), stop=False)
                            nc.tensor.matmul(ps1, lhsT=wb,
                                             rhs=xr[:, K1F + k, ns],
                                             start=False, stop=False)
                        nc.tensor.matmul(
                            ps0, lhsT=w1[0:32, K1F, ja * 128:(ja + 1) * 128],
                            rhs=xr[0:32, 12, ns], start=False, stop=True)
                        nc.tensor.matmul(
                            ps1, lhsT=w1[0:32, K1F, jb_ * 128:(jb_ + 1) * 128],
                            rhs=xr[0:32, 12, ns], start=False, stop=True)
                        nc.scalar.activation(
                            out=a1[:, ja // 2, ja % 2, ns], in_=ps0,
                            func=ACTF.Sign, bias=nt1[:, ja:ja + 1], scale=1.0)
                        nc.scalar.activation(
                            out=a1[:, jb_ // 2, jb_ % 2, ns], in_=ps1,
                            func=ACTF.Sign, bias=nt1[:, jb_:jb_ + 1], scale=1.0)

            # ---- fc2/fc3 ----
            with tc.tile_pool(name="a2p", bufs=1) as a2p:
                a2 = a2p.tile([128, NT3, 2, BC], FP8)

                # fc2: fp8 DoubleRow + sign threshold
                with tc.tile_pool(name="w2s", bufs=2) as w2s:
                    for jb in range(NJ2 // JB):
                        wt = w2s.tile([128, NT2, 2, JB * 128], FP8, tag="w2t")
                        for t in range(NT2):
                            nc.sync.dma_start(out=wt[:, t], in_=w2p[jb, t])
                        for j in range(JB):
                            jj = jb * JB + j
                            for n in range(NB):
                                ns = slice(n * 512, (n + 1) * 512)
                                ps = psum.tile([128, 512], F32, tag="ps")
                                for t in range(NT2):
                                    nc.tensor.matmul(
                                        ps,
                                        lhsT=wt[:, t, :, j * 128:(j + 1) * 128],
                                        rhs=a1[:, t, :, ns],
                                        start=(t == 0),
                                        stop=(t == NT2 - 1),
                                        perf_mode=DR,
                                    )
                                nc.scalar.activation(
                                    out=a2[:, jj // 2, jj % 2, ns], in_=ps,
                                    func=ACTF.Sign, bias=nt2[:, jj:jj + 1],
                                    scale=1.0)

                # fc3 (fp8 DoubleRow) + bn3/hardtanh + fused col-tiled fc4
                with (
                    tc.tile_pool(name="w3s", bufs=3) as w3s,
                    tc.tile_pool(name="h3p", bufs=6) as h3p,
                ):
                    for jb in range(NJ3 // JB):
                        wt = w3s.tile([128, NT3, 2, JB * 128], FP8, tag="w3t")
                        for tg in range(NT3 // 2):
                            nc.sync.dma_start(
                                out=wt[:, 2 * tg:2 * tg + 2],
                                in_=w3p[jb, 2 * tg:2 * tg + 2].rearrange(
                                    "t p i n -> p t i n"),
                            )
                        h3s = []
                        for j in range(JB):
                            jj = jb * JB + j
                            h3 = h3p.tile([128, BC], BF16, tag="h3")
                            for n in range(NB):
                                ns = slice(n * 512, (n + 1) * 512)
                                ps = psum.tile([128, 512], F32, tag="ps")
                                for t in range(NT3):
                                    nc.tensor.matmul(
                                        ps,
                                        lhsT=wt[:, t, :, j * 128:(j + 1) * 128],
                                        rhs=a2[:, t, :, ns],
                                        start=(t == 0),
                                        stop=(t == NT3 - 1),
                                        perf_mode=DR,
                                    )
                                tmp = h3p.tile([128, 512], F32, tag="bn3tmp")
                                nc.scalar.activation(
                                    out=tmp, in_=ps, func=ACTF.Identity,
                                    bias=c3s[:, jj:jj + 1],
                                    scale=s3s[:, jj:jj + 1])
                                nc.vector.tensor_scalar(
                                    out=h3[:, ns], in0=tmp,
                                    scalar1=-1.0, scalar2=1.0,
                                    op0=ALU.max, op1=ALU.min)
                            h3s.append(h3)
                        # fused fc4, batched: the 4 M=16 matmuls of a slab
                        # run concurrently on 4 PE column groups
                        for n in range(NB):
                            ns = slice(n * 512, (n + 1) * 512)
                            for j in range(JB):
                                jj = jb * JB + j
                                nc.tensor.matmul(
                                    lg[n][32 * j:32 * j + 2 * NCLS, :],
                                    lhsT=w4s[:, jj, :],
                                    rhs=h3s[j][:, ns],
                                    start=False,
                                    stop=(jb == NJ3 // JB - 1),
                                    skip_group_check=True,
                                    tile_position=(0, 32 * j),
                                )

            # ---- epilogue: stripe-sum, +b4, transpose, log_softmax ----
            with tc.tile_pool(name="epi", bufs=2) as epi:
                ident = consts.tile([NCLS, NCLS], F32)
                make_identity(nc, ident)
                lsb = epi.tile([NCLS, BC], F32, tag="lsb")
                for n in range(NB):
                    # sum the 4 class-group (x hi/lo) stripes: selection
                    # matmul in full fp32
                    cp = epi.tile([128, 512], F32, tag="cp")
                    nc.scalar.copy(out=cp, in_=lg[n])
                    nc.tensor.matmul(
                        lg[n][0:NCLS, :], lhsT=selm, rhs=cp,
                        start=True, stop=True, skip_group_check=True)
                    nc.scalar.activation(
                        out=lsb[:, n * 512:(n + 1) * 512],
                        in_=lg[n][0:NCLS, :],
                        func=ACTF.Identity,
                        bias=b4s[:, 0:1],
                        scale=1.0,
                    )
                tp = psum_tp.tile([128, NBCH, NCLS], F32)
                for b in range(NBCH):
                    nc.tensor.transpose(
                        tp[:, b, :], lsb[:, b * 128:(b + 1) * 128], ident)
                # log_softmax without max-shift: logits are O(5), exp safe
                ex = epi.tile([128, NBCH, 10], F32, tag="ex")
                nc.scalar.activation(out=ex, in_=tp[:, :, 0:10], func=ACTF.Exp)
                sm = epi.tile([128, NBCH], F32, tag="sm")
                nc.vector.tensor_reduce(
                    out=sm, in_=ex, axis=mybir.AxisListType.X, op=ALU.add)
                lnt = epi.tile([128, NBCH], F32, tag="lnt")
                nc.scalar.activation(out=lnt, in_=sm, func=ACTF.Ln)
                res = epi.tile([128, NBCH, 10], F32, tag="res")
                for b in range(NBCH):
                    nc.vector.tensor_scalar(
                        out=res[:, b, :], in0=tp[:, b, 0:10],
                        scalar1=lnt[:, b:b + 1],
                        scalar2=None, op0=ALU.subtract,
                    )
                nc.sync.dma_start(
                    out=out.rearrange("(b p) c -> p b c", p=128), in_=res)

    _legalize_multiwait(nc)
    return nc


def _prep_inputs(inputs):
    f64 = {k: np.asarray(v, np.float64) for k, v in inputs.items()
           if k != "x"}
    x = np.asarray(inputs["x"], np.float32)

    s1 = f64["g1"] / np.sqrt(f64["v1"] + EPS)
    t1 = f64["m1"] - f64["b1"] - f64["be1"] / s1
    s2 = f64["g2"] / np.sqrt(f64["v2"] + EPS)
    t2 = f64["m2"] - f64["b2"] - f64["be2"] / s2
    s3 = f64["g3"] / np.sqrt(f64["v3"] + EPS)
    c3 = (f64["b3"] - f64["m3"]) * s3 + f64["be3"]

    shared = {}
    # cvec [128, 24+48*3+16]: per-feature consts arranged [partition, tile];
    # last 16 cols: selection matrix summing the 4 class-group stripes and
    # their hi/lo halves (rows 32g+c and 32g+16+c -> column c)
    cvec = np.zeros((128, NJ1 + 3 * NJ3 + NCLS), np.float32)
    cvec[:, 0:NJ1] = (-t1).astype(np.float32).reshape(NJ1, 128).T
    cvec[:, NJ1:NJ1 + NJ3] = (-t2).astype(np.float32).reshape(NJ3, 128).T
    cvec[:, NJ1 + NJ3:NJ1 + 2 * NJ3] = s3.astype(np.float32).reshape(NJ3, 128).T
    cvec[:, NJ1 + 2 * NJ3:NJ1 + 3 * NJ3] = (
        c3.astype(np.float32).reshape(NJ3, 128).T)
    for g in range(4):
        for c in range(NCLS):
            cvec[32 * g + c, NJ1 + 3 * NJ3 + c] = 1.0
            cvec[32 * g + NCLS + c, NJ1 + 3 * NJ3 + c] = 1.0
    shared["cvec"] = np.ascontiguousarray(cvec)

    b4p = np.zeros((NCLS, 1), np.float32)
    b4p[:10, 0] = np.asarray(inputs["b4"], np.float32)
    shared["b4c"] = b4p

    # w1: sign, transposed to [in, out]; slots 0..5 = rows 0..767, slot 6 =
    # the 16 remainder rows replicated 2x (hi pass at partitions 0:16, lo
    # pass at 16:32) + zeros; fp8 on the wire, f32r after on-device convert
    w1b = np.sign(np.asarray(inputs["w1"], np.float32)).astype(np.float32)
    w1T = w1b.T  # [784, D1]
    w1arr = np.zeros((K1F + 1, 128, D1), np.float32)
    w1arr[:K1F] = w1T[:768].reshape(K1F, 128, D1)
    for p in range(2):
        w1arr[K1F, 16 * p:16 * (p + 1)] = w1T[768:784]
    shared["w1t"] = np.ascontiguousarray(w1arr.astype(NP_FP8))

    # w2/w3: sign -> DoubleRow pair layout [njb, nt, 128, 2, JB*128] fp8
    def pack_dr(w, njb_out):
        wT = np.sign(np.asarray(w, np.float32)).T  # [in, out]
        nin, nout = wT.shape
        nt = nin // 256
        a = wT.reshape(nt, 2, 128, nout).transpose(0, 2, 1, 3)  # [nt,128,2,out]
        a = a.reshape(nt, 128, 2, njb_out, JB * 128).transpose(3, 0, 1, 2, 4)
        return np.ascontiguousarray(a.astype(NP_FP8))

    shared["w2p"] = pack_dr(inputs["w2"], NJ2 // JB)
    shared["w3p"] = pack_dr(inputs["w3"], NJ3 // JB)

    # w4: [10, D2] -> [128, NJ3*32] bf16: slot j cols 0:16 = hi(w4) chunk,
    # cols 16:32 = lo = bf16(w4 - hi); element [k, j*32+h*16+c] covers
    # w4[c, j*128+k]. The hi+lo split keeps fp32-level accuracy; the
    # selection matmul sums both halves of all 4 stripes.
    import ml_dtypes
    w4 = np.asarray(inputs["w4"], np.float32)
    w4tp = np.zeros((D2, NCLS), np.float32)
    w4tp[:, :10] = w4.T
    w4h = w4tp.astype(ml_dtypes.bfloat16)
    w4l = (w4tp - w4h.astype(np.float32)).astype(ml_dtypes.bfloat16)
    w4all = np.zeros((128, NJ3, 2 * NCLS), ml_dtypes.bfloat16)
    w4all[:, :, 0:NCLS] = w4h.reshape(NJ3, 128, NCLS).transpose(1, 0, 2)
    w4all[:, :, NCLS:] = w4l.reshape(NJ3, 128, NCLS).transpose(1, 0, 2)
    shared["w4t"] = np.ascontiguousarray(w4all.reshape(128, -1))

    # x: transpose, split into hi (RNE to 12 significand bits -- the
    # HW-measured width of the fp32r read path, so hi passes unharmed)
    # + lo (remainder, <=12 bits, residual ~2^-21 rel, below fp32 noise);
    # tiles 0..5 = hi full k-tiles, 6..11 = lo, 12 packs both passes'
    # remainder rows 768..783 (+ zero pad)
    xT = np.ascontiguousarray(x.T)  # [784, B]
    m, e = np.frexp(xT.astype(np.float64))
    xh = np.ldexp(np.rint(m * 4096.0) / 4096.0, e).astype(np.float32)
    xl = xT - xh
    per_core = []
    for c in range(NCORES):
        sl = slice(c * BC, (c + 1) * BC)
        xrt = np.zeros((NXT, 128, BC), np.float32)
        xrt[0:K1F] = xh[:768, sl].reshape(K1F, 128, BC)
        xrt[K1F:2 * K1F] = xl[:768, sl].reshape(K1F, 128, BC)
        xrt[12, 0:16] = xh[768:784, sl]
        xrt[12, 16:32] = xl[768:784, sl]
        m_ = dict(shared)
        m_["xrt"] = np.ascontiguousarray(xrt)
        per_core.append(m_)
    return per_core


_NC_CACHE = None


def kernel(**inputs):
    global _NC_CACHE, LAST_EXEC_NS
    if _NC_CACHE is None:
        _NC_CACHE = _build_nc()
    nc = _NC_CACHE
    in_maps = _prep_inputs(inputs)
    kwargs = {}
    if TRACE:
        _install_ntff_shim()
        kwargs = dict(trace=True, tmpdir=TRACE_DIR)
    res = None
    outs = None
    for attempt in range(3):
        try:
            res = run_bass_kernel_spmd(nc, in_maps, core_ids=list(range(NCORES)),
                                       **kwargs)
            outs = [np.asarray(res.results[c]["out"]) for c in range(NCORES)]
            break
        except Exception:
            if attempt == 2:
                raise
    LAST_EXEC_NS = res.exec_time_ns
    return np.concatenate(outs, axis=0)


def _install_ntff_shim():
    """antenv.axon_hooks shim so trace=True works under axon (profiling only)."""
    import contextlib
    import ctypes
    import types

    if "antenv.axon_hooks" in sys.modules:
        return
    try:
        lib = ctypes.CDLL("/opt/axon/libaxon_pjrt.so")
        lib.axon_start_nrt_profile.argtypes = [
            ctypes.POINTER(ctypes.c_int64), ctypes.c_size_t]
        lib.axon_start_nrt_profile.restype = ctypes.c_int64
        lib.axon_stop_nrt_profile.argtypes = [ctypes.c_char_p]
        lib.axon_stop_nrt_profile.restype = ctypes.c_int64
    except (OSError, AttributeError):
        return

    @contextlib.contextmanager
    def _hook(output_dir, device_ids):
        import jax
        jax.devices()
        if device_ids:
            ids = (ctypes.c_int64 * len(device_ids))(*device_ids)
            rc = lib.axon_start_nrt_profile(ids, len(device_ids))
        else:
            rc = lib.axon_start_nrt_profile(None, 0)
        if rc != 0:
            raise RuntimeError(f"axon_start_nrt_profile rc={rc}")
        try:
            yield
        finally:
            n = lib.axon_stop_nrt_profile(str(output_dir).encode())
            print(f"ntff: {n} profile file(s) -> {output_dir}", file=sys.stderr)

    mod = types.ModuleType("antenv.axon_hooks")
    mod.get_axon_ntff_profile_hook = lambda: _hook
    mod.set_axon_ntff_profile_hook = lambda h: None
    sys.modules["antenv.axon_hooks"] = mod
